# revision 8
# baseline (speedup 1.0000x reference)
"""Bass/Trainium2 kernel for nn_Attentioncell (Bahdanau-style attention cell).

Mathematical simplification (rel-err ~6e-7 vs the jax reference): the
per-step scores are
    scores[b,l] = (total[b,l,:] + (h @ W2)[b,:]) @ V
               = (total @ V)[b,l] + (h @ W2 @ V)[b]
and softmax over l is invariant to the per-b shift, so the attention
weights are identical for every timestep and independent of h:
    attn = softmax_l(x_static @ (W1 @ V))        (b2, W2, h0 drop out)
    ctx[b,:] = sum_l attn[b,l] * x_static[b,l,:]
    out[b,t,:] = x[b,t,:] @ W3[:D] + ctx[b,:] @ W3[D:] + b3

The scan disappears entirely; the kernel is a handful of matmuls and a
softmax, data-parallel over batch B=32 across 8 NeuronCores (4 per core).

v2 restructure (from the v1 trace: vector-bound scores phase, PE work
queued behind the ctx chain, 4us serial tail):
  - scores via one fused scalar_tensor_tensor (mul + free-axis accum)
    per chunk instead of separate mul + reduce instructions.
  - w1v broadcast on-chip (gpsimd partition_broadcast) instead of
    shipping a 114KB host-broadcast copy.
  - x@W3top matmuls interleaved between ctx matmuls so the PE consumes
    them during DMA gaps instead of after the ctx chain.
  - 1/Z and b3 folded into one fused c2 normalize step; the final
    out = psum + c2 add runs on DVE against a partition-broadcast c2
    (no trailing PE matmuls).
  - output shipped bf16 and cast to f32 on host (halves out DMA).
  - w3b blocks ordered last in the DMA schedule, consumed block-by-
    block by the c2 matmuls as they land.
"""

import numpy as np

B, T, L, S, D = 32, 32, 196, 512, 512
NCORES = 8
BLOC = B // NCORES          # 4 batches per core
BT = BLOC * T               # 128 output rows per core
BL = BLOC * L               # 784 static rows per core
NCH = 7                     # bl chunks
CH = BL // NCH              # 112 rows per chunk
MW = NCH * BLOC             # mask width (28)
XSW = MW + NCH * S          # xsp width: [mask | chunks 0..6]

_cache = {}


def _build_graph():
    import concourse.bacc as bacc
    import concourse.tile as tile
    from concourse import mybir

    f32 = mybir.dt.float32
    bf16 = mybir.dt.bfloat16
    mult = mybir.AluOpType.mult
    add = mybir.AluOpType.add
    nc = bacc.Bacc("TRN2", target_bir_lowering=False, debug=False,
                   num_devices=NCORES)

    xs_d = nc.dram_tensor("xsp", [CH, XSW], bf16, kind="ExternalInput").ap()
    # w3tx packs [xt | w3t blocks]
    w3t_d = nc.dram_tensor("w3tx", [128, 512 + 4 * D], bf16,
                           kind="ExternalInput").ap()
    w3b_d = nc.dram_tensor("w3b", [128, 4 * D], bf16, kind="ExternalInput").ap()
    # smalls packs [id4 | ind4 | w1v] (bf16); b3r4 is f32 [4, D]
    sm_d = nc.dram_tensor("smalls", [4, 4 + BT + S], bf16,
                          kind="ExternalInput").ap()
    b3_d = nc.dram_tensor("b3r4", [BLOC, D], f32, kind="ExternalInput").ap()
    out_d = nc.dram_tensor("out", [BT, D], bf16, kind="ExternalOutput").ap()

    with tile.TileContext(nc) as tc:
        with (
            tc.tile_pool(name="big", bufs=1) as big,
            tc.tile_pool(name="small", bufs=1) as small,
            tc.tile_pool(name="scratch", bufs=2) as scratch,
            tc.tile_pool(name="ps_acc", bufs=1, space="PSUM") as ps_acc,
            tc.tile_pool(name="ps_tr", bufs=2, space="PSUM") as ps_tr,
        ):
            xsp = big.tile([CH, XSW], bf16, tag="xsp")
            mask = xsp[:, 0:MW]
            xs = xsp[:, MW:]
            w3tx = big.tile([128, 512 + 4 * D], bf16, tag="w3tx")
            xt = w3tx[:, 0:512]
            w3t = w3tx[:, 512:]
            w3b = big.tile([128, 4 * D], bf16, tag="w3b")
            sm = small.tile([4, 4 + BT + S], bf16, tag="sm")
            id4 = sm[:, 0:4]
            ind4 = sm[:, 4:4 + BT]
            w1v = sm[0:1, 4 + BT:]
            b3r4 = small.tile([BLOC, D], f32, tag="b3r4")
            w1vb = big.tile([CH, S], bf16, tag="w1vb")
            scores = small.tile([CH, NCH], f32, tag="scores")
            etile = small.tile([CH, NCH], bf16, tag="etile")
            E = small.tile([CH, MW], bf16, tag="E")
            esum = small.tile([CH, BLOC], f32, tag="esum")
            ones = small.tile([CH, 1], f32, tag="ones")
            recipZ = small.tile([BLOC, 1], f32, tag="recipZ")
            ctx_sb = small.tile([BLOC, S], bf16, tag="ctx_sb")
            ctxT = small.tile([128, 4 * BLOC], bf16, tag="ctxT")
            c2n = small.tile([BLOC, D], bf16, tag="c2n")
            out_sb = big.tile([BT, D], bf16, tag="out_sb")

            # ---- DMA schedule. 3 issuing engines = 3 HW queues sharing
            # 16 DMA engines (~250GB/s aggregate). Within each queue,
            # tensors are ordered by when compute needs them; w3b blocks
            # go last (consumed block-wise by the c2 matmuls). ----
            # sync queue: score-chain chunks 0..3 (mask rides with c0c1)
            nc.sync.dma_start(xsp[:, 0:MW + 2 * S], xs_d[:, 0:MW + 2 * S])
            nc.scalar.dma_start(sm[:], sm_d[:])
            nc.scalar.dma_start(b3r4[:], b3_d[:])
            nc.sync.dma_start(xsp[:, MW + 2 * S:MW + 4 * S],
                              xs_d[:, MW + 2 * S:MW + 4 * S])
            nc.scalar.dma_start(xsp[:, MW + 4 * S:MW + 6 * S],
                                xs_d[:, MW + 4 * S:MW + 6 * S])
            nc.scalar.dma_start(xsp[:, MW + 6 * S:], xs_d[:, MW + 6 * S:])
            nc.gpsimd.dma_start(w3tx[:, 0:1536], w3t_d[:, 0:1536])
            nc.gpsimd.dma_start(w3tx[:, 1536:], w3t_d[:, 1536:])
            nc.sync.dma_start(w3b[:, 0:D], w3b_d[:, 0:D])
            nc.scalar.dma_start(w3b[:, D:2 * D], w3b_d[:, D:2 * D])
            nc.sync.dma_start(w3b[:, 2 * D:3 * D], w3b_d[:, 2 * D:3 * D])
            nc.scalar.dma_start(w3b[:, 3 * D:], w3b_d[:, 3 * D:])

            nc.vector.memset(ones[:], 1.0)
            # broadcast w1v row across the 112 chunk partitions on-chip
            nc.gpsimd.partition_broadcast(w1vb[:], w1v, channels=CH)

            out_ps = ps_acc.tile([BT, D], f32, tag="out_ps")
            ctx_ps = ps_acc.tile([BLOC, S], f32, tag="ctx_ps")
            z_ps = ps_acc.tile([BLOC, 1], f32, tag="z_ps")

            def scores_chunk(c):
                # fused mul + free-axis accumulate in one DVE op
                prod = scratch.tile([CH, S], bf16, tag="prod")
                nc.vector.scalar_tensor_tensor(
                    prod[:], xs[:, c * S:(c + 1) * S], 1.0, w1vb[:],
                    op0=mult, op1=mult, accum_out=scores[:, c:c + 1])

            def e_chunk(c):
                nc.scalar.activation(etile[:, c:c + 1], scores[:, c:c + 1],
                                     mybir.ActivationFunctionType.Exp)
                nc.vector.tensor_mul(
                    E[:, c * BLOC:(c + 1) * BLOC].rearrange(
                        "p (c b) -> p c b", b=BLOC),
                    etile[:, c:c + 1].to_broadcast((CH, 1, BLOC)),
                    mask[:, c * BLOC:(c + 1) * BLOC].rearrange(
                        "p (c b) -> p c b", b=BLOC),
                )

            def ctx_mm(c):
                nc.tensor.matmul(ctx_ps[:], E[:, c * BLOC:(c + 1) * BLOC],
                                 xs[:, c * S:(c + 1) * S],
                                 start=(c == 0), stop=(c == NCH - 1))

            def xt_mm(j):
                nc.tensor.matmul(out_ps[:], xt[:, j * 128:(j + 1) * 128],
                                 w3t[:, j * D:(j + 1) * D],
                                 start=(j == 0), stop=False,
                                 skip_group_check=True)

            # scores/E/ctx chunks paced by DMA arrival; xt matmuls
            # interleave so the PE eats them in the gaps.
            for c in range(NCH):
                scores_chunk(c)
                e_chunk(c)
                ctx_mm(c)
                if c < 4:
                    xt_mm(c)

            # Z per batch: pre-sum E over chunks on DVE (strided view),
            # then one tiny [112,4]^T @ ones matmul.
            nc.vector.tensor_reduce(
                esum[:],
                E[:].rearrange("p (c b) -> p b c", b=BLOC),
                axis=mybir.AxisListType.X,
                op=mybir.AluOpType.add)
            nc.tensor.matmul(z_ps[:], esum[:], ones[:], start=True, stop=True)
            nc.vector.reciprocal(recipZ[:], z_ps[:])
            nc.vector.tensor_copy(ctx_sb[:], ctx_ps[:])

            # ---- transpose ctx ([4,512] -> 4x [128,4]) on PE ----
            for j in range(4):
                tr = ps_tr.tile([128, BLOC], bf16, tag="tr")
                nc.tensor.transpose(tr[:], ctx_sb[:, j * 128:(j + 1) * 128],
                                    id4[:])
                nc.vector.tensor_copy(ctxT[:, j * BLOC:(j + 1) * BLOC], tr[:])

            # ---- c2 = ctx @ W3bot (unnormalized) ----
            c2_ps = ps_acc.tile([BLOC, D], f32, tag="c2_ps")
            for j in range(4):
                nc.tensor.matmul(c2_ps[:], ctxT[:, j * BLOC:(j + 1) * BLOC],
                                 w3b[:, j * D:(j + 1) * D],
                                 start=(j == 0), stop=(j == 3))
            # c2n = c2/Z + b3, one fused DVE op (b3 folded here, so the
            # final accumulation needs only a 4-row indicator matmul)
            nc.vector.scalar_tensor_tensor(
                c2n[:], c2_ps[:], recipZ[:], b3r4[:], op0=mult, op1=add)

            # ---- out += Ind4^T @ c2n, in two row halves so the
            # copy-out + DMA of half 0 overlaps the matmul of half 1 ----
            H = BT // 2
            for h in range(2):
                sl = slice(h * H, (h + 1) * H)
                nc.tensor.matmul(out_ps[sl, :], ind4[:, sl], c2n[:],
                                 start=False, stop=(h == 1),
                                 skip_group_check=True)
                nc.vector.tensor_copy(out_sb[sl, :], out_ps[sl, :])
                eng = nc.sync if h == 0 else nc.scalar
                eng.dma_start(out_d[sl, :], out_sb[sl, :])

    nc.compile()
    return nc


def _get_graph():
    if "nc" not in _cache:
        _cache["nc"] = _build_graph()
    return _cache["nc"]


def _consts():
    if "consts" in _cache:
        return _cache["consts"]
    mask = np.zeros((CH, NCH, BLOC), np.float32)
    for c in range(NCH):
        for p in range(CH):
            mask[p, c, (c * CH + p) // L] = 1.0
    _cache["consts"] = {"_mask": mask.reshape(CH, NCH * BLOC)}
    return _cache["consts"]


def kernel(x, x_static, h0, W1, W2, W3, b2, b3, V, **_unused):
    import ml_dtypes
    from concourse.bass_utils import run_bass_kernel_spmd
    bf = ml_dtypes.bfloat16

    x = np.asarray(x, np.float32)
    x_static = np.asarray(x_static, np.float32)
    W1 = np.asarray(W1, np.float32)
    W3 = np.asarray(W3, np.float32)
    b3 = np.asarray(b3, np.float32)
    V = np.asarray(V, np.float32)

    # Host-side weight folding (weights are per-model constants).
    w1v = (W1 @ V).reshape(1, -1).astype(np.float32)        # [1, S]
    # per-partition-contiguous permuted layouts (one big DMA segment
    # per partition):
    w3t = (W3[:D].reshape(4, 128, D).transpose(1, 0, 2)
           .reshape(128, 4 * D))
    w3b = np.ascontiguousarray(
        W3[D:].reshape(4, 128, D).transpose(1, 0, 2).reshape(128, 4 * D)
        .astype(bf))
    sm = np.zeros((4, 4 + BT + S), np.float32)
    sm[0:4, 0:4] = np.eye(4)
    for b in range(BLOC):
        sm[b, 4 + b * T:4 + (b + 1) * T] = 1.0
    sm[0:1, 4 + BT:] = w1v
    sm = np.ascontiguousarray(sm.astype(bf))
    b3r4 = np.ascontiguousarray(
        np.broadcast_to(b3.reshape(1, D), (BLOC, D)).astype(np.float32))
    consts = _consts()

    nc = _get_graph()
    in_maps = []
    for i in range(NCORES):
        sl = slice(i * BLOC, (i + 1) * BLOC)
        xs_l = x_static[sl].reshape(BL, S)
        xs_p = xs_l.reshape(NCH, CH, S).transpose(1, 0, 2).reshape(CH, NCH * S)
        xsp = np.ascontiguousarray(
            np.concatenate([consts["_mask"], xs_p], axis=1).astype(bf))
        xt_l = x[sl].reshape(BT, D).T                        # [512, 128]
        xt_p = (xt_l.reshape(4, 128, 128).transpose(1, 0, 2)
                .reshape(128, 512))
        w3tx = np.ascontiguousarray(
            np.concatenate([xt_p, w3t], axis=1).astype(bf))
        in_maps.append({
            "xsp": xsp, "w3tx": w3tx, "w3b": w3b,
            "smalls": sm, "b3r4": b3r4,
        })
    res = run_bass_kernel_spmd(nc, in_maps, core_ids=list(range(NCORES)))
    out = np.empty((B, T, D), np.float32)
    for i in range(NCORES):
        out[i * BLOC:(i + 1) * BLOC] = (
            res.results[i]["out"].astype(np.float32).reshape(BLOC, T, D))
    return out


# revision 10
# speedup vs baseline: 1.1799x; 1.1799x over previous
"""Bass/Trainium2 kernel for nn_Attentioncell (Bahdanau-style attention cell).

Mathematical simplification (rel-err ~6e-7 vs the jax reference): the
per-step scores are
    scores[b,l] = (total[b,l,:] + (h @ W2)[b,:]) @ V
               = (total @ V)[b,l] + (h @ W2 @ V)[b]
and softmax over l is invariant to the per-b shift, so the attention
weights are identical for every timestep and independent of h:
    attn = softmax_l(x_static @ (W1 @ V))        (b2, W2, h0 drop out)
    ctx[b,:] = sum_l attn[b,l] * x_static[b,l,:]
    out[b,t,:] = x[b,t,:] @ W3[:D] + ctx[b,:] @ W3[D:] + b3

The scan disappears entirely; the kernel is a handful of matmuls and a
softmax, data-parallel over batch B=32 across 8 NeuronCores (4 per core).

v3 scheduling model (measured): the three DMA queues (sync/scalar/
gpsimd engines) share 16 HW DMA engines at ~253GB/s aggregate;
dependency tracking is per-SBUF-tile at queue-position granularity, so
every input gets its own tile and consumers fire as soon as their own
transfer lands.  Key structure:
  - scores via one fused scalar_tensor_tensor (mul + free-axis accum)
    per chunk on DVE; exp on ACT; E-mask build on GpSimd; ctx matmul
    per chunk on PE -- a 4-engine pipeline paced by chunk DMA arrival.
  - w1v broadcast across partitions via a K=1 PE matmul into PSUM
    (read directly by the DVE), nothing extra on the wire.
  - x@W3top matmuls fill PE gaps during the DMA phase; W3bot blocks are
    scheduled last on the wire and consumed block-wise by the c2
    matmuls; 1/Z and b3 fold into one fused c2 normalize op.
  - ctx->ctxT copy and the final psum->sbuf casts split ACT/DVE into
    separate tiles so the halves run in parallel.
  - output shipped bf16 and cast to f32 on host (halves out DMA).
"""

import numpy as np

B, T, L, S, D = 32, 32, 196, 512, 512
NCORES = 8
BLOC = B // NCORES          # 4 batches per core
BT = BLOC * T               # 128 output rows per core
BL = BLOC * L               # 784 static rows per core
NCH = 7                     # bl chunks
CH = BL // NCH              # 112 rows per chunk
MW = NCH * BLOC             # mask width (28)
XSW = MW + NCH * S          # xsp width: [mask | chunks 0..6]

_cache = {}


def _build_graph():
    import concourse.bacc as bacc
    import concourse.tile as tile
    from concourse import mybir

    f32 = mybir.dt.float32
    bf16 = mybir.dt.bfloat16
    mult = mybir.AluOpType.mult
    add = mybir.AluOpType.add
    nc = bacc.Bacc("TRN2", target_bir_lowering=False, debug=False,
                   num_devices=NCORES)

    xs_d = nc.dram_tensor("xsp", [CH, XSW], bf16, kind="ExternalInput").ap()
    # w3tx packs [xt | w3t blocks]
    w3t_d = nc.dram_tensor("w3tx", [128, 512 + 4 * D], bf16,
                           kind="ExternalInput").ap()
    w3b_d = nc.dram_tensor("w3b", [128, 4 * D], bf16, kind="ExternalInput").ap()
    # smalls packs [id4 | ind4 | w1v] (bf16); b3r4 is f32 [4, D]
    sm_d = nc.dram_tensor("smalls", [4, 4 + BT + S], bf16,
                          kind="ExternalInput").ap()
    b3_d = nc.dram_tensor("b3r4", [BLOC, D], f32, kind="ExternalInput").ap()
    out_d = nc.dram_tensor("out", [BT, D], bf16, kind="ExternalOutput").ap()

    with tile.TileContext(nc) as tc:
        with (
            tc.tile_pool(name="big", bufs=1) as big,
            tc.tile_pool(name="small", bufs=1) as small,
            tc.tile_pool(name="scratch", bufs=2) as scratch,
            tc.tile_pool(name="ps_acc", bufs=1, space="PSUM") as ps_acc,
            tc.tile_pool(name="ps_tr", bufs=2, space="PSUM") as ps_tr,
        ):
            # one tile per DMA so consumers wait only on their own bytes
            m0 = big.tile([CH, MW + S], bf16, tag="m0")
            mask = m0[:, 0:MW]
            x12 = big.tile([CH, 2 * S], bf16, tag="x12")
            x34 = big.tile([CH, 2 * S], bf16, tag="x34")
            x56 = big.tile([CH, 2 * S], bf16, tag="x56")

            def xs_c(c):
                if c == 0:
                    return m0[:, MW:]
                t = (x12, x34, x56)[(c - 1) // 2]
                o = ((c - 1) % 2) * S
                return t[:, o:o + S]

            xt = big.tile([128, 512], bf16, tag="xt")
            w3t01 = big.tile([128, 2 * D], bf16, tag="w3t01")
            w3t23 = big.tile([128, 2 * D], bf16, tag="w3t23")

            def w3t_j(j):
                t = w3t01 if j < 2 else w3t23
                return t[:, (j % 2) * D:(j % 2) * D + D]

            w3bt = [big.tile([128, D], bf16, tag=f"w3b{j}", name=f"w3b{j}")
                    for j in range(4)]
            sm = small.tile([4, 4 + BT + S], bf16, tag="sm")
            id4 = sm[:, 0:4]
            ind4 = sm[:, 4:4 + BT]
            w1v = sm[0:1, 4 + BT:]
            b3r4 = small.tile([BLOC, D], f32, tag="b3r4")
            onesrow = small.tile([1, CH], bf16, tag="onesrow")
            ones = small.tile([CH, 1], f32, tag="ones")
            scores = small.tile([CH, NCH], f32, tag="scores")
            etile = small.tile([CH, NCH], bf16, tag="etile")
            E = small.tile([CH, MW], bf16, tag="E")
            esum = small.tile([CH, BLOC], f32, tag="esum")
            recipZ = small.tile([BLOC, 1], f32, tag="recipZ")
            ctx_sbA = small.tile([BLOC, 2 * 128], bf16, tag="ctx_sbA")
            ctx_sbB = small.tile([BLOC, 2 * 128], bf16, tag="ctx_sbB")
            ctxT = small.tile([128, 4 * BLOC], bf16, tag="ctxT")
            c2n = small.tile([BLOC, D], bf16, tag="c2n")
            out_sbA = big.tile([BT // 2, D], bf16, tag="out_sbA")
            out_sbB = big.tile([BT // 2, D], bf16, tag="out_sbB")

            # ---- DMA schedule: 3 queues, bytes ordered by need;
            # W3bot blocks last, consumed block-wise by the c2 matmuls.
            nc.sync.dma_start(sm[:], sm_d[:])
            nc.scalar.dma_start(b3r4[:], b3_d[:])
            nc.sync.dma_start(m0[:], xs_d[:, 0:MW + S])
            nc.scalar.dma_start(x12[:], xs_d[:, MW + S:MW + 3 * S])
            nc.gpsimd.dma_start(w3t01[:], w3t_d[:, 512:512 + 2 * D])
            nc.sync.dma_start(x34[:], xs_d[:, MW + 3 * S:MW + 5 * S])
            nc.scalar.dma_start(x56[:], xs_d[:, MW + 5 * S:])
            nc.gpsimd.dma_start(xt[:], w3t_d[:, 0:512])
            nc.gpsimd.dma_start(w3t23[:], w3t_d[:, 512 + 2 * D:])
            nc.sync.dma_start(w3bt[0][:], w3b_d[:, 0:D])
            nc.scalar.dma_start(w3bt[1][:], w3b_d[:, D:2 * D])
            nc.sync.dma_start(w3bt[2][:], w3b_d[:, 2 * D:3 * D])
            nc.scalar.dma_start(w3bt[3][:], w3b_d[:, 3 * D:])

            nc.vector.memset(ones[:], 1.0)
            nc.vector.memset(onesrow[:], 1.0)

            # broadcast w1v across the 112 chunk partitions via a K=1
            # matmul; the DVE reads the result straight from PSUM.
            w1v_ps = ps_acc.tile([CH, S], f32, tag="w1v_ps")
            nc.tensor.matmul(w1v_ps[:], onesrow[:], w1v, start=True, stop=True)

            out_ps = ps_acc.tile([BT, D], f32, tag="out_ps")
            ctx_ps = ps_acc.tile([BLOC, S], f32, tag="ctx_ps")
            z_ps = ps_acc.tile([BLOC, 1], f32, tag="z_ps")

            def scores_chunk(c):
                # fused mul + free-axis accumulate in one DVE op
                prod = scratch.tile([CH, S], bf16, tag="prod")
                nc.vector.scalar_tensor_tensor(
                    prod[:], xs_c(c), 1.0, w1v_ps[:],
                    op0=mult, op1=mult, accum_out=scores[:, c:c + 1])

            def e_chunk(c):
                nc.scalar.activation(etile[:, c:c + 1], scores[:, c:c + 1],
                                     mybir.ActivationFunctionType.Exp)
                nc.gpsimd.tensor_mul(
                    E[:, c * BLOC:(c + 1) * BLOC].rearrange(
                        "p (c b) -> p c b", b=BLOC),
                    etile[:, c:c + 1].to_broadcast((CH, 1, BLOC)),
                    mask[:, c * BLOC:(c + 1) * BLOC].rearrange(
                        "p (c b) -> p c b", b=BLOC),
                )

            def ctx_mm(c):
                nc.tensor.matmul(ctx_ps[:], E[:, c * BLOC:(c + 1) * BLOC],
                                 xs_c(c), start=(c == 0), stop=(c == NCH - 1))

            def xt_mm(j):
                nc.tensor.matmul(out_ps[:], xt[:, j * 128:(j + 1) * 128],
                                 w3t_j(j), start=(j == 0), stop=False,
                                 skip_group_check=True)

            # scores/E/ctx chunks paced by DMA arrival; xt matmuls
            # interleave so the PE eats them in the gaps.
            for c in range(NCH):
                scores_chunk(c)
                e_chunk(c)
                ctx_mm(c)
                if c < 4:
                    xt_mm(c)

            # Z per batch: pre-sum E over chunks on DVE (strided view),
            # then one tiny [112,4]^T @ ones matmul.
            nc.vector.tensor_reduce(
                esum[:],
                E[:].rearrange("p (c b) -> p b c", b=BLOC),
                axis=mybir.AxisListType.X,
                op=mybir.AluOpType.add)
            nc.tensor.matmul(z_ps[:], esum[:], ones[:], start=True, stop=True)
            nc.vector.reciprocal(recipZ[:], z_ps[:])
            # psum->sbuf ctx copy split across DVE/ACT so halves overlap
            nc.vector.tensor_copy(ctx_sbA[:], ctx_ps[:, 0:256])
            nc.scalar.copy(ctx_sbB[:], ctx_ps[:, 256:])

            # ---- transpose ctx ([4,512] -> 4x [128,4]) on PE ----
            for j in range(4):
                src = ctx_sbA if j < 2 else ctx_sbB
                tr = ps_tr.tile([128, BLOC], bf16, tag="tr")
                nc.tensor.transpose(
                    tr[:], src[:, (j % 2) * 128:(j % 2) * 128 + 128], id4[:])
                nc.vector.tensor_copy(ctxT[:, j * BLOC:(j + 1) * BLOC], tr[:])

            # ---- c2 = ctx @ W3bot (unnormalized) ----
            c2_ps = ps_acc.tile([BLOC, D], f32, tag="c2_ps")
            for j in range(4):
                nc.tensor.matmul(c2_ps[:], ctxT[:, j * BLOC:(j + 1) * BLOC],
                                 w3bt[j][:], start=(j == 0), stop=(j == 3))
            # c2n = c2/Z + b3, one fused DVE op (b3 folded here, so the
            # final accumulation needs only a 4-row indicator matmul)
            nc.vector.scalar_tensor_tensor(
                c2n[:], c2_ps[:], recipZ[:], b3r4[:], op0=mult, op1=add)

            # ---- out += Ind4^T @ c2n: both halves back-to-back on PE,
            # then the psum->sbuf casts run in parallel on DVE/ACT ----
            H = BT // 2
            for h in range(2):
                sl = slice(h * H, (h + 1) * H)
                nc.tensor.matmul(out_ps[sl, :], ind4[:, sl], c2n[:],
                                 start=False, stop=(h == 1),
                                 skip_group_check=True)
            nc.vector.tensor_copy(out_sbA[:], out_ps[0:H, :])
            nc.scalar.copy(out_sbB[:], out_ps[H:, :])
            nc.sync.dma_start(out_d[0:H, :], out_sbA[:])
            nc.scalar.dma_start(out_d[H:, :], out_sbB[:])

    nc.compile()
    return nc


def _get_graph():
    if "nc" not in _cache:
        _cache["nc"] = _build_graph()
    return _cache["nc"]


def _consts():
    if "consts" in _cache:
        return _cache["consts"]
    mask = np.zeros((CH, NCH, BLOC), np.float32)
    for c in range(NCH):
        for p in range(CH):
            mask[p, c, (c * CH + p) // L] = 1.0
    _cache["consts"] = {"_mask": mask.reshape(CH, NCH * BLOC)}
    return _cache["consts"]


def kernel(x, x_static, h0, W1, W2, W3, b2, b3, V, **_unused):
    import ml_dtypes
    from concourse.bass_utils import run_bass_kernel_spmd
    bf = ml_dtypes.bfloat16

    x = np.asarray(x, np.float32)
    x_static = np.asarray(x_static, np.float32)
    W1 = np.asarray(W1, np.float32)
    W3 = np.asarray(W3, np.float32)
    b3 = np.asarray(b3, np.float32)
    V = np.asarray(V, np.float32)

    # Host-side weight folding (weights are per-model constants).
    w1v = (W1 @ V).reshape(1, -1).astype(np.float32)        # [1, S]
    # per-partition-contiguous permuted layouts (one big DMA segment
    # per partition):
    w3t = (W3[:D].reshape(4, 128, D).transpose(1, 0, 2)
           .reshape(128, 4 * D))
    w3b = np.ascontiguousarray(
        W3[D:].reshape(4, 128, D).transpose(1, 0, 2).reshape(128, 4 * D)
        .astype(bf))
    sm = np.zeros((4, 4 + BT + S), np.float32)
    sm[0:4, 0:4] = np.eye(4)
    for b in range(BLOC):
        sm[b, 4 + b * T:4 + (b + 1) * T] = 1.0
    sm[0:1, 4 + BT:] = w1v
    sm = np.ascontiguousarray(sm.astype(bf))
    b3r4 = np.ascontiguousarray(
        np.broadcast_to(b3.reshape(1, D), (BLOC, D)).astype(np.float32))
    consts = _consts()

    nc = _get_graph()
    in_maps = []
    for i in range(NCORES):
        sl = slice(i * BLOC, (i + 1) * BLOC)
        xs_l = x_static[sl].reshape(BL, S)
        xs_p = xs_l.reshape(NCH, CH, S).transpose(1, 0, 2).reshape(CH, NCH * S)
        xsp = np.ascontiguousarray(
            np.concatenate([consts["_mask"], xs_p], axis=1).astype(bf))
        xt_l = x[sl].reshape(BT, D).T                        # [512, 128]
        xt_p = (xt_l.reshape(4, 128, 128).transpose(1, 0, 2)
                .reshape(128, 512))
        w3tx = np.ascontiguousarray(
            np.concatenate([xt_p, w3t], axis=1).astype(bf))
        in_maps.append({
            "xsp": xsp, "w3tx": w3tx, "w3b": w3b,
            "smalls": sm, "b3r4": b3r4,
        })
    res = run_bass_kernel_spmd(nc, in_maps, core_ids=list(range(NCORES)))
    out = np.empty((B, T, D), np.float32)
    for i in range(NCORES):
        out[i * BLOC:(i + 1) * BLOC] = (
            res.results[i]["out"].astype(np.float32).reshape(BLOC, T, D))
    return out
